# revision 23
# baseline (speedup 1.0000x reference)
import sys
sys.path.insert(0, "/opt/trn_rl_repo")
import numpy as np
import concourse.bass as bass
import concourse.mybir as mybir
import concourse.tile as tile
from concourse import bacc
from concourse.bass_utils import run_bass_kernel_spmd
from concourse.masks import make_identity

F32 = mybir.dt.float32
BF16 = mybir.dt.bfloat16
AF = mybir.ActivationFunctionType
OP = mybir.AluOpType
AX = mybir.AxisListType

S = 2048          # sequence length
H = 4096          # hidden dim
DH = 128          # head dim
NQ = 4            # q heads per core (32 / 8)
NT = S // 128     # 16 q tiles of 128
NCORES = 8
SCALE = 1.0 / np.sqrt(128.0)
NEG = -1.0e33

_CACHED = {}


def _phase_b(nc, tc, perm, hidT_d, wqkvT_d, qT, kT, vT, cosb, sinb):
    with tc.tile_pool(name="wqp", bufs=1) as wq_p, \
         tc.tile_pool(name="hid", bufs=2) as hid_p, \
         tc.tile_pool(name="bps", bufs=2, space="PSUM") as bps, \
         tc.tile_pool(name="rt", bufs=2) as rt_p:
        wqs = []
        for kt in range(32):
            w = wq_p.tile([128, 768], BF16, tag=f"wq{kt}")
            nc.sync.dma_start(w, wqkvT_d[kt * 128:(kt + 1) * 128, :])
            wqs.append(w)

        for sc in range(4):
            ssl = slice(sc * 512, (sc + 1) * 512)
            hids = []
            for kt in range(32):
                ht = hid_p.tile([128, 512], BF16, tag=f"h{kt}")
                eng = nc.gpsimd if kt % 2 == 0 else nc.scalar
                eng.dma_start(ht, hidT_d[kt * 128:(kt + 1) * 128, ssl])
                hids.append(ht)
            if sc == 0:
                # pre-warm exp table AFTER chunk-0 issues so the ~2.7us
                # table load doesn't delay the first hid tiles
                warm = rt_p.tile([128, 1], BF16, tag="warm")
                nc.scalar.activation(warm, cosb[:, 0:1], AF.Exp)
            for m in range(6):
                ps = bps.tile([128, 512], F32, tag="bacc")
                for kt in range(32):
                    nc.tensor.matmul(
                        ps, wqs[kt][:, m * 128:(m + 1) * 128], hids[kt],
                        start=(kt == 0), stop=(kt == 31))
                if m < 5:
                    dst = qT[:, m, ssl] if m < 4 else kT[:, ssl]
                    t1 = rt_p.tile([128, 512], F32, tag="t1")
                    t2 = rt_p.tile([128, 512], F32, tag="t2")
                    nc.vector.tensor_mul(t1, ps, cosb[:, ssl])
                    nc.vector.tensor_mul(t2[0:64], ps[64:128], sinb[0:64, ssl])
                    nc.vector.tensor_mul(t2[64:128], ps[0:64], sinb[64:128, ssl])
                    nc.vector.tensor_add(dst, t1, t2)
                else:
                    nc.vector.tensor_copy(vT[:, ssl], ps)


def _phase_c(nc, tc, qT, kT, vT, vnat, attnT, identb, dmask):
    # PSUM banks: scores 2x[128,1024] (4) + tp 2x[128,8,128]bf16 (2)
    #             + pv 2x[128,4,128] (2) = 8
    with tc.tile_pool(name="cps", bufs=2, space="PSUM") as sc_p, \
         tc.tile_pool(name="tps", bufs=2, space="PSUM") as tr_p, \
         tc.tile_pool(name="pvs", bufs=2, space="PSUM") as pv_p, \
         tc.tile_pool(name="pt", bufs=2) as pt_p, \
         tc.tile_pool(name="psb", bufs=8) as psb_p, \
         tc.tile_pool(name="dn", bufs=2) as dn_p:
        # V natural tiles from v^T strip
        for g in range(2):
            tp = tr_p.tile([128, 8, 128], BF16, tag="tp")
            for i in range(8):
                st8 = 8 * g + i
                nc.tensor.transpose(
                    tp[:, i, :], vT[:, st8 * 128:(st8 + 1) * 128], identb)
            nc.vector.tensor_copy(vnat[:, 8 * g:8 * g + 8, :], tp)

        def scores_exp(t):
            klen = (t + 1) * 128
            nch = (klen + 1023) // 1024
            denp = dn_p.tile([128, NQ, 2], F32, tag="denp")
            pts = []
            for h in range(NQ):
                pth = pt_p.tile([128, S], BF16, tag=f"pt{h}")
                pts.append(pth)
                for ch in range(nch):
                    c0 = ch * 1024
                    cl = min(1024, klen - c0)
                    st = sc_p.tile([128, 1024], F32, tag="sc")
                    for j in range(0, cl, 512):
                        jl = min(512, cl - j)
                        nc.tensor.matmul(
                            st[:, j:j + jl],
                            qT[:, h, t * 128:(t + 1) * 128],
                            kT[:, c0 + j:c0 + j + jl],
                            start=True, stop=True)
                    if ch == nch - 1:
                        nc.vector.tensor_add(
                            st[:, cl - 128:cl], st[:, cl - 128:cl], dmask)
                    nc.scalar.activation(
                        pth[:, c0:c0 + cl], st[:, 0:cl], AF.Exp,
                        scale=SCALE, accum_out=denp[:, h, ch:ch + 1])
            return pts, denp, nch

        def norm_pv(t, pts, denp, nch):
            klen = (t + 1) * 128
            recs = dn_p.tile([128, NQ], F32, tag="recs")
            for h in range(NQ):
                if nch == 1:
                    nc.vector.reciprocal(recs[:, h:h + 1], denp[:, h, 0:1])
                else:
                    den = dn_p.tile([128, 1], F32, tag="den")
                    nc.vector.reduce_sum(den, denp[:, h, 0:nch], axis=AX.X)
                    nc.vector.reciprocal(recs[:, h:h + 1], den)
                nc.vector.tensor_scalar_mul(
                    pts[h][:, 0:klen], pts[h][:, 0:klen], recs[:, h:h + 1])
            # all transposes first (copies overlap), then all PV chains
            nb = t + 1
            psbs = {}
            for h in range(NQ):
                for g0 in range(0, nb, 8):
                    ge = min(8, nb - g0)
                    tp = tr_p.tile([128, 8, 128], BF16, tag="tp")
                    for i in range(ge):
                        kb = g0 + i
                        nc.tensor.transpose(
                            tp[:, i, :],
                            pts[h][:, kb * 128:(kb + 1) * 128], identb)
                    psb = psb_p.tile([128, 8, 128], BF16, tag="psb")
                    nc.vector.tensor_copy(psb[:, 0:ge, :], tp[:, 0:ge, :])
                    psbs[(h, g0)] = psb
            pv = pv_p.tile([128, NQ, 128], F32, tag="pv")
            for h in range(NQ):
                for g0 in range(0, nb, 8):
                    ge = min(8, nb - g0)
                    psb = psbs[(h, g0)]
                    for i in range(ge):
                        kb = g0 + i
                        nc.tensor.matmul(
                            pv[:, h, :], vnat[:, kb, :], psb[:, i, :],
                            start=(kb == 0), stop=(kb == nb - 1))
            nc.vector.tensor_copy(attnT[:, :, t * 128:(t + 1) * 128], pv)

        # 1-stage software pipeline: scores/exp of t+1 run on PE while
        # the exp->recip->normalize chain of t drains on ACT/DVE
        prev = None
        for t in range(NT):
            cur = (t, *scores_exp(t))
            if prev is not None:
                norm_pv(*prev)
            prev = cur
        norm_pv(*prev)


def _phase_d(nc, tc, wogs, attnT, out_d):
    with tc.tile_pool(name="dps", bufs=4, space="PSUM") as dps, \
         tc.tile_pool(name="ob", bufs=4) as ob_p:
        for m in range(32):
            mg, mo = divmod(m, 4)
            for scc in range(4):
                po = dps.tile([128, 512], F32, tag="po")
                for a in range(NQ):
                    nc.tensor.matmul(
                        po, wogs[mg][:, a, mo * 128:(mo + 1) * 128],
                        attnT[:, a, scc * 512:(scc + 1) * 512],
                        start=(a == 0), stop=(a == NQ - 1))
                ob = ob_p.tile([128, 512], BF16, tag="ob")
                if (m * 4 + scc) % 2 == 0:
                    nc.vector.tensor_copy(ob, po)
                    oeng = nc.sync
                else:
                    nc.scalar.copy(ob, po)
                    oeng = nc.scalar
                oeng.dma_start(
                    out_d[m * 128:(m + 1) * 128, scc * 512:(scc + 1) * 512],
                    ob)


def _build_nc():
    nc = bacc.Bacc(None, target_bir_lowering=False, debug=False)
    # Inputs host-pre-transposed/cast so no PE transposes are needed:
    #   hidt  = hidden[0].T            [H, S]
    #   wqkvt = [Wq_c; Wk_c; Wv_c].T   [H, 768]   (cols 0:512 q, 512:640 k, 640:768 v)
    #   wot   = Wo[:, c*512:...].T     [512, H]
    #   cos/sin [d=128, S], sin sign-folded for rotate_half
    hidT_d = nc.dram_tensor("hidt", [H, S], BF16, kind="ExternalInput")
    wqkvT_d = nc.dram_tensor("wqkvt", [H, 768], BF16, kind="ExternalInput")
    woT_d = nc.dram_tensor("wot", [NQ * DH, H], BF16, kind="ExternalInput")
    cos_d = nc.dram_tensor("cos", [DH, S], BF16, kind="ExternalInput")
    sin_d = nc.dram_tensor("sin", [DH, S], BF16, kind="ExternalInput")
    out_d = nc.dram_tensor("outt", [H, S], BF16, kind="ExternalOutput")

    with tile.TileContext(nc) as tc:
        with tc.tile_pool(name="perm", bufs=1) as perm:
            identf = perm.tile([128, 128], F32, tag="identf")
            make_identity(nc, identf)
            identb = perm.tile([128, 128], BF16, tag="identb")
            nc.vector.tensor_copy(identb, identf)
            # additive causal mask for the diagonal 128x128 block:
            # 0 where k <= q, NEG where k > q  (q = partition, k = free)
            zeros = perm.tile([128, 128], F32, tag="zeros")
            nc.gpsimd.memset(zeros, 0.0)
            dmask = perm.tile([128, 128], F32, tag="dmask")
            nc.gpsimd.affine_select(
                out=dmask, in_=zeros, pattern=[[-1, 128]],
                compare_op=OP.is_ge, fill=NEG,
                base=0, channel_multiplier=1)

            # persistent strips (bf16)
            qT = perm.tile([128, NQ, S], BF16, tag="qT")
            kT = perm.tile([128, S], BF16, tag="kT")
            vT = perm.tile([128, S], BF16, tag="vT")
            vnat = perm.tile([128, NT, 128], BF16, tag="vnat")
            attnT = perm.tile([128, NQ, S], BF16, tag="attnT")
            cosb = perm.tile([128, S], BF16, tag="cosb")
            sinb = perm.tile([128, S], BF16, tag="sinb")
            nc.sync.dma_start(cosb, cos_d[:, :])
            nc.sync.dma_start(sinb, sin_d[:, :])

            _phase_b(nc, tc, perm, hidT_d, wqkvT_d, qT, kT, vT, cosb, sinb)

            # o_proj weights: load early on the (now idle) sync queue
            with tc.tile_pool(name="wo", bufs=1) as wo_p:
                wogs = []
                for mg in range(8):
                    wg = wo_p.tile([128, NQ, 512], BF16, tag=f"wo{mg}")
                    for a in range(NQ):
                        nc.sync.dma_start(
                            wg[:, a, :],
                            woT_d[a * 128:(a + 1) * 128, mg * 512:(mg + 1) * 512])
                    wogs.append(wg)

                _phase_c(nc, tc, qT, kT, vT, vnat, attnT, identb, dmask)
                _phase_d(nc, tc, wogs, attnT, out_d)
    nc.compile()
    return nc


def _prep_inputs(hidden_states, position_ids, Wq, Wk, Wv, Wo):
    bf16 = np.dtype(mybir.dt.np(BF16))
    hs = np.asarray(hidden_states, dtype=np.float32)
    hidT = np.ascontiguousarray(hs[0].T).astype(bf16)

    pos = np.asarray(position_ids).reshape(-1).astype(np.float64)
    invf = 1.0 / (10000.0 ** (np.arange(0, 128, 2, dtype=np.float64) / 128.0))
    ang = invf[:, None] * pos[None, :]
    cos_t = np.concatenate([np.cos(ang), np.cos(ang)], axis=0).astype(bf16)
    sin_t = np.concatenate([-np.sin(ang), np.sin(ang)], axis=0).astype(bf16)

    Wq = np.asarray(Wq, dtype=np.float32)
    Wk = np.asarray(Wk, dtype=np.float32)
    Wv = np.asarray(Wv, dtype=np.float32)
    Wo = np.asarray(Wo, dtype=np.float32)
    in_maps = []
    for c in range(NCORES):
        wqkv = np.concatenate([
            Wq[c * 512:(c + 1) * 512],
            Wk[c * 128:(c + 1) * 128],
            Wv[c * 128:(c + 1) * 128]], axis=0)          # [768, H]
        wqkvT = np.ascontiguousarray(wqkv.T).astype(bf16)  # [H, 768]
        woT = np.ascontiguousarray(Wo[:, c * 512:(c + 1) * 512].T).astype(bf16)
        in_maps.append({"hidt": hidT, "wqkvt": wqkvT, "wot": woT,
                        "cos": cos_t, "sin": sin_t})
    return in_maps


def kernel(hidden_states, position_ids, Wq, Wk, Wv, Wo, **extra):
    hs = np.asarray(hidden_states)
    B = hs.shape[0]
    assert B == 1 and hs.shape[1] == S and hs.shape[2] == H

    if "nc" not in _CACHED:
        _CACHED["nc"] = _build_nc()
    nc = _CACHED["nc"]

    in_maps = _prep_inputs(hidden_states, position_ids, Wq, Wk, Wv, Wo)
    res = run_bass_kernel_spmd(nc, in_maps, core_ids=list(range(NCORES)))
    out = np.zeros((H, S), dtype=np.float32)
    for c in range(NCORES):
        out += np.asarray(res.results[c]["outt"]).astype(np.float32)
    return np.ascontiguousarray(out.T).reshape(1, S, H)
